# revision 1
# baseline (speedup 1.0000x reference)
"""8-core Trainium2 Bass kernel for nn_MetabolicGNN (GCN x2 + GAT + MLP).

Strategy: nodes permuted into 392 degree-balanced tiles of 128 (49 tiles/core);
edges grouped by dst tile, padded to 17 chunks of 128 per tile. Per layer each
core computes its shard of the node-feature table, AllGathers the full table,
then aggregates its tiles: per 128-edge chunk an indirect-DMA gather pulls
src rows, a one-hot [edge x dst] matrix built on DVE feeds a PE matmul that
accumulates into PSUM. GCN normalization is folded: table rows pre-scaled by
dinv[src], output rows scaled by dinv[dst]. GAT attention terms a_s ride in
the gather table (fp32 words in a bf16 row); a_d is broadcast to edges via a
PE transpose of the one-hot; exp() without segment-max (logits are in
[-0.9, 4.3] for this input distribution, mathematically identical).
"""
import sys

sys.path.insert(0, "/opt/trn_rl_repo")

import numpy as np

N = 50000
E = 800000
IN_DIM, HID, OUT_DIM, HEADS = 256, 128, 64, 4
NCORES = 8
P = 128
TPC = 49                    # tiles per core
NT = NCORES * TPC           # 392 tiles
N_PAD = NT * P              # 50176
NPC = TPC * P               # 6272 nodes per core
CPT = 17                    # chunks per tile (padded; max tile load is 2172)
CHUNKS = TPC * CPT          # 833 chunks per core
GD = HEADS * HID            # 512
GROW = 528                  # GAT table row: 512 bf16 xwg + 4 f32 a_s (8 slots) + pad


def _preprocess(edge_index):
    src = edge_index[0].astype(np.int64)
    dst = edge_index[1].astype(np.int64)
    loop = np.arange(N, dtype=np.int64)
    srcA = np.concatenate([src, loop])
    dstA = np.concatenate([dst, loop])
    deg = np.bincount(dstA, minlength=N).astype(np.int64)
    dinv = (1.0 / np.sqrt(deg)).astype(np.float32)

    # degree-balanced assignment of nodes to NT tiles of exactly P slots
    import heapq
    order = np.argsort(-deg, kind="stable")
    tile_load = np.zeros(NT, dtype=np.int64)
    tile_fill = np.zeros(NT, dtype=np.int64)
    node_tile = np.empty(N_PAD, dtype=np.int64)
    node_slot = np.empty(N_PAD, dtype=np.int64)
    heap = [(0, t) for t in range(NT)]
    heapq.heapify(heap)
    for n in order:
        while True:
            load, t = heapq.heappop(heap)
            if tile_fill[t] < P:
                break
        node_tile[n] = t
        node_slot[n] = tile_fill[t]
        tile_fill[t] += 1
        tile_load[t] = load + deg[n]
        if tile_fill[t] < P:
            heapq.heappush(heap, (tile_load[t], t))
    free = [(t, s) for t in range(NT) for s in range(tile_fill[t], P)]
    for pid, (t, s) in zip(range(N, N_PAD), free):
        node_tile[pid] = t
        node_slot[pid] = s
    assert tile_load.max() <= CPT * P, tile_load.max()

    perm = node_tile * P + node_slot            # old id -> new id

    e_tile = node_tile[dstA]
    e_slot = node_slot[dstA]
    e_srcnew = perm[srcA]
    eo = np.argsort(e_tile, kind="stable")
    e_tile, e_slot, e_srcnew = e_tile[eo], e_slot[eo], e_srcnew[eo]
    starts = np.searchsorted(e_tile, np.arange(NT))
    ends = np.searchsorted(e_tile, np.arange(NT) + 1)

    epc = CPT * P
    src_idx = np.zeros((NCORES, TPC * epc), dtype=np.int32)
    dst_slot = np.full((NCORES, TPC * epc), -1.0, dtype=np.float32)
    for t in range(NT):
        c, tl = divmod(t, TPC)
        s, e = starts[t], ends[t]
        base = tl * epc
        src_idx[c, base:base + (e - s)] = e_srcnew[s:e]
        dst_slot[c, base:base + (e - s)] = e_slot[s:e]

    dinv_new = np.ones(N_PAD, dtype=np.float32)
    dinv_new[perm[:N]] = dinv
    return src_idx, dst_slot, dinv_new, perm


def _build_nc():
    import concourse.bass as bass
    import concourse.bacc as bacc
    import concourse.tile as tile
    from concourse import mybir

    f32 = mybir.dt.float32
    bf16 = mybir.dt.bfloat16
    i32 = mybir.dt.int32
    AF = mybir.ActivationFunctionType
    OP = mybir.AluOpType
    AX = mybir.AxisListType

    nc = bacc.Bacc(trn_type="TRN2", target_bir_lowering=False, num_devices=NCORES,
                   dynamic_dma_scratch_size=65536, num_swdge_queues=4)

    # ---- I/O ----
    x_c = nc.dram_tensor("x_c", [NPC, IN_DIM], f32, kind="ExternalInput")
    idxsrc_d = nc.dram_tensor("idxsrc", [P, CHUNKS], i32, kind="ExternalInput")
    dstslot_d = nc.dram_tensor("dstslot", [P, CHUNKS], f32, kind="ExternalInput")
    dinv_d = nc.dram_tensor("dinv_t", [P, TPC], f32, kind="ExternalInput")
    iota_d = nc.dram_tensor("iota_f", [P, P], f32, kind="ExternalInput")
    ident_d = nc.dram_tensor("ident", [P, P], f32, kind="ExternalInput")
    win_d = nc.dram_tensor("Win", [IN_DIM, HID], f32, kind="ExternalInput")
    bin_d = nc.dram_tensor("bin_pp", [P, 1], f32, kind="ExternalInput")
    wg1_d = nc.dram_tensor("Wg1", [HID, HID], f32, kind="ExternalInput")
    wg2_d = nc.dram_tensor("Wg2", [HID, HID], f32, kind="ExternalInput")
    bg1_d = nc.dram_tensor("bg1_bc", [P, HID], f32, kind="ExternalInput")
    bg2_d = nc.dram_tensor("bg2_bc", [P, HID], f32, kind="ExternalInput")
    g1g_d = nc.dram_tensor("g1g_bc", [P, HID], f32, kind="ExternalInput")
    g1b_d = nc.dram_tensor("g1b_bc", [P, HID], f32, kind="ExternalInput")
    g2g_d = nc.dram_tensor("g2g_bc", [P, HID], f32, kind="ExternalInput")
    g2b_d = nc.dram_tensor("g2b_bc", [P, HID], f32, kind="ExternalInput")
    wgat_d = nc.dram_tensor("Wgat", [HID, GD], f32, kind="ExternalInput")
    vsvd_d = nc.dram_tensor("VsVd", [HID, 2 * HEADS], f32, kind="ExternalInput")
    watt_d = nc.dram_tensor("Watt", [GD, HID], f32, kind="ExternalInput")
    batt_d = nc.dram_tensor("batt_pp", [P, 1], f32, kind="ExternalInput")
    wout_d = nc.dram_tensor("Wout", [HID, OUT_DIM], f32, kind="ExternalInput")
    bout_d = nc.dram_tensor("bout_bc", [P, OUT_DIM], f32, kind="ExternalInput")
    eps_d = nc.dram_tensor("eps_pp", [P, 1], f32, kind="ExternalInput")
    out_c = nc.dram_tensor("out_c", [NPC, OUT_DIM], f32, kind="ExternalOutput")

    # ---- internal DRAM (collectives) ----
    ag_in1 = nc.dram_tensor("ag_in1", [NPC, HID], bf16, kind="Internal")
    tb1 = nc.dram_tensor("tb1", [N_PAD, HID], bf16, kind="Internal", addr_space="Shared")
    ag_in2 = nc.dram_tensor("ag_in2", [NPC, HID], bf16, kind="Internal")
    tb2 = nc.dram_tensor("tb2", [N_PAD, HID], bf16, kind="Internal", addr_space="Shared")
    ag_in3 = nc.dram_tensor("ag_in3", [NPC, GROW], bf16, kind="Internal")
    tb3 = nc.dram_tensor("tb3", [N_PAD, GROW], bf16, kind="Internal", addr_space="Shared")

    rg = [list(range(NCORES))]

    with tile.TileContext(nc) as tc:
        with (
            tc.tile_pool(name="const", bufs=1) as cpool,
            tc.tile_pool(name="big", bufs=1) as bigpool,
            tc.tile_pool(name="gath", bufs=24) as gpool,
            tc.tile_pool(name="oh", bufs=8) as ohpool,
            tc.tile_pool(name="work", bufs=4) as wpool,
            tc.tile_pool(name="small", bufs=10) as spool,
            tc.tile_pool(name="ps", bufs=2, space="PSUM") as pspool,
            tc.tile_pool(name="pst", bufs=2, space="PSUM") as tppool,
        ):
            # ---------- constants ----------
            def cload(dram, shape, dtype=f32):
                t = cpool.tile(shape, dtype, tag="c_" + dram.name)
                nc.sync.dma_start(out=t[:], in_=dram[:])
                return t

            iota_t = cload(iota_d, [P, P])
            ident_t = cload(ident_d, [P, P])
            win_t = cpool.tile([P, IN_DIM // P, HID], f32, tag='c_Win')
            nc.sync.dma_start(out=win_t[:],
                              in_=win_d[:].rearrange("(h p) c -> p h c", p=P))
            bin_t = cload(bin_d, [P, 1])
            wg1_t = cload(wg1_d, [P, HID])
            wg2_t = cload(wg2_d, [P, HID])
            bg1_t = cload(bg1_d, [P, HID])
            bg2_t = cload(bg2_d, [P, HID])
            g1g_t = cload(g1g_d, [P, HID])
            g1b_t = cload(g1b_d, [P, HID])
            g2g_t = cload(g2g_d, [P, HID])
            g2b_t = cload(g2b_d, [P, HID])
            wgat_t = cload(wgat_d, [P, GD])
            vsvd_t = cload(vsvd_d, [P, 2 * HEADS])
            watt_t = cpool.tile([P, GD // P, HID], f32, tag='c_Watt')
            nc.sync.dma_start(out=watt_t[:],
                              in_=watt_d[:].rearrange("(k p) c -> p k c", p=P))
            batt_t = cload(batt_d, [P, 1])
            wout_t = cload(wout_d, [P, OUT_DIM])
            bout_t = cload(bout_d, [P, OUT_DIM])
            eps_t = cload(eps_d, [P, 1])
            dinv_t = cload(dinv_d, [P, TPC])
            identbf = cpool.tile([P, P], bf16, tag="c_identbf")
            nc.vector.tensor_copy(out=identbf[:], in_=ident_t[:])
            idxsrc = cload(idxsrc_d, [P, CHUNKS], i32)
            dstslot = cload(dstslot_d, [P, CHUNKS])

            h0T = bigpool.tile([P, NPC], f32, tag="h0T")

            # ---------- P1: h0T = relu(Win.T @ x.T + bin), feature-major ----------
            NCH = NPC // 512  # 12.25 -> handle 12 full + 1 partial below
            for ch in range(13):
                n0 = ch * 512
                nn = min(512, NPC - n0)
                nsub = nn // P
                xT = wpool.tile([P, 2, 512], f32, tag="xT")
                for s in range(nsub):
                    xt = wpool.tile([P, IN_DIM], f32, tag="xload")
                    nc.sync.dma_start(out=xt[:], in_=x_c[n0 + s * P:n0 + (s + 1) * P, :])
                    for h in range(2):
                        tp = tppool.tile([P, P], f32, tag="tp")
                        nc.tensor.transpose(out=tp[:], in_=xt[:, h * P:(h + 1) * P],
                                            identity=ident_t[:])
                        nc.vector.tensor_copy(out=xT[:, h, s * P:(s + 1) * P], in_=tp[:])
                hp = pspool.tile([P, 512], f32, tag="mm")
                for h in range(2):
                    nc.tensor.matmul(out=hp[:, :nn], lhsT=win_t[:, h, :], rhs=xT[:, h, :nn],
                                     start=(h == 0), stop=(h == 1))
                nc.scalar.activation(out=h0T[:, n0:n0 + nn], in_=hp[:, :nn],
                                     func=AF.Relu, bias=bin_t[:], scale=1.0)

            # ---------- helper: xw table build + AG ----------
            def build_table(srcT, w_t, ag_in, tb, scale_dinv):
                for t in range(TPC):
                    ps = pspool.tile([P, HID], f32, tag="mm")
                    nc.tensor.matmul(out=ps[:], lhsT=srcT[:, t * P:(t + 1) * P],
                                     rhs=w_t[:], start=True, stop=True)
                    sb = wpool.tile([P, HID], bf16, tag="xwsb")
                    if scale_dinv:
                        nc.vector.tensor_scalar_mul(out=sb[:], in0=ps[:],
                                                    scalar1=dinv_t[:, t:t + 1])
                    else:
                        nc.vector.tensor_copy(out=sb[:], in_=ps[:])
                    nc.sync.dma_start(out=ag_in[t * P:(t + 1) * P, :], in_=sb[:])
                nc.gpsimd.collective_compute(
                    "AllGather", OP.bypass, ins=[ag_in[:]], outs=[tb[:]],
                    replica_groups=rg)

            # ---------- helper: layernorm(+relu) on node-major tile ----------
            def ln_relu(dst, src, gam, bet):
                st = spool.tile([P, 6], f32, tag="lnst")
                nc.vector.bn_stats(out=st[:], in_=src[:])
                mv = spool.tile([P, 2], f32, tag="lnmv")
                nc.vector.bn_aggr(out=mv[:], in_=st[:])
                rstd = spool.tile([P, 1], f32, tag="lnrs")
                nc.scalar.activation(out=rstd[:], in_=mv[:, 1:2], func=AF.Sqrt,
                                     bias=eps_t[:], scale=1.0)
                nc.vector.reciprocal(out=rstd[:], in_=rstd[:])
                nc.vector.tensor_scalar(out=src[:], in0=src[:], scalar1=mv[:, 0:1],
                                        scalar2=rstd[:], op0=OP.subtract, op1=OP.mult)
                nc.vector.tensor_mul(out=src[:], in0=src[:], in1=gam[:])
                nc.vector.tensor_add(out=src[:], in0=src[:], in1=bet[:])
                nc.scalar.activation(out=dst[:], in_=src[:], func=AF.Relu)

            # ---------- helper: GCN aggregation pass ----------
            def gcn_pass(tb, bg_t, gam, bet, outT, resT):
                for t in range(TPC):
                    U = pspool.tile([P, HID], f32, tag="U1")
                    gts = []
                    for j in range(CPT):
                        c = t * CPT + j
                        gt = gpool.tile([P, HID], bf16, tag="gcng")
                        ii = nc.gpsimd.indirect_dma_start(
                            out=gt[:], out_offset=None, in_=tb[:],
                            in_offset=bass.IndirectOffsetOnAxis(
                                ap=idxsrc[:, c:c + 1], axis=0))
                        if j % 4:
                            ii.ins.queue = f"qPoolDynamic{j % 4}"
                        gts.append(gt)
                    for j in range(CPT):
                        c = t * CPT + j
                        oh = ohpool.tile([P, P], bf16, tag="gcnoh")
                        nc.vector.tensor_tensor(
                            out=oh[:], in0=dstslot[:, c:c + 1].to_broadcast([P, P]),
                            in1=iota_t[:], op=OP.is_equal)
                        nc.tensor.matmul(out=U[:], lhsT=oh[:], rhs=gts[j][:],
                                         start=(j == 0), stop=(j == CPT - 1))
                    pre = wpool.tile([P, HID], f32, tag="gcnpre")
                    nc.vector.scalar_tensor_tensor(
                        out=pre[:], in0=U[:], scalar=dinv_t[:, t:t + 1], in1=bg_t[:],
                        op0=OP.mult, op1=OP.add)
                    nm = wpool.tile([P, HID], f32, tag="gcnnm")
                    ln_relu(nm, pre, gam, bet)
                    tp = tppool.tile([P, P], f32, tag="tp")
                    nc.tensor.transpose(out=tp[:], in_=nm[:], identity=ident_t[:])
                    if resT is None:
                        nc.vector.tensor_copy(out=outT[:, t * P:(t + 1) * P], in_=tp[:])
                    else:
                        nc.vector.tensor_add(out=outT[:, t * P:(t + 1) * P],
                                             in0=resT[:, t * P:(t + 1) * P], in1=tp[:])

            # ---------- GCN layer 1 ----------
            build_table(h0T, wg1_t, ag_in1, tb1, True)
            x1T = bigpool.tile([P, NPC], f32, tag="x1T")
            gcn_pass(tb1, bg1_t, g1g_t, g1b_t, x1T, None)

            # ---------- GCN layer 2 (residual) ----------
            build_table(x1T, wg2_t, ag_in2, tb2, True)
            x2T = bigpool.tile([P, NPC], f32, tag="h0T")  # reuse h0T slot
            gcn_pass(tb2, bg2_t, g2g_t, g2b_t, x2T, x1T)

            # ---------- P6: GAT table ----------
            a_d_all = cpool.tile([P, TPC * HEADS], f32, tag='c_adall')
            for t in range(TPC):
                ps = pspool.tile([P, GD], f32, tag="mm")
                nc.tensor.matmul(out=ps[:], lhsT=x2T[:, t * P:(t + 1) * P],
                                 rhs=wgat_t[:], start=True, stop=True)
                ps8 = tppool.tile([P, 2 * HEADS], f32, tag="tp")
                nc.tensor.matmul(out=ps8[:], lhsT=x2T[:, t * P:(t + 1) * P],
                                 rhs=vsvd_t[:], start=True, stop=True)
                stg = wpool.tile([P, GROW], bf16, tag="stg")
                nc.vector.tensor_copy(out=stg[:, 0:GD], in_=ps[:])
                stgf = stg[:, GD:GD + 8].bitcast(f32)
                nc.vector.tensor_copy(out=stgf, in_=ps8[:, 0:HEADS])
                nc.vector.tensor_copy(out=a_d_all[:, t * HEADS:(t + 1) * HEADS],
                                      in_=ps8[:, HEADS:2 * HEADS])
                nc.sync.dma_start(out=ag_in3[t * P:(t + 1) * P, :], in_=stg[:])
            nc.gpsimd.collective_compute(
                "AllGather", OP.bypass, ins=[ag_in3[:]], outs=[tb3[:]],
                replica_groups=rg)

            # ---------- P7: GAT aggregation + att_out ----------
            aoutT = bigpool.tile([P, NPC], f32, tag="x1T")  # reuse x1T slot
            for t in range(TPC):
                U1 = pspool.tile([P, 258], f32, tag="U1")
                U2 = pspool.tile([P, 258], f32, tag="U2")
                gts = []
                for j in range(CPT):
                    c = t * CPT + j
                    gt = gpool.tile([P, GROW], bf16, tag="gatg")
                    ii = nc.gpsimd.indirect_dma_start(
                        out=gt[:], out_offset=None, in_=tb3[:],
                        in_offset=bass.IndirectOffsetOnAxis(
                            ap=idxsrc[:, c:c + 1], axis=0))
                    if j % 4:
                        ii.ins.queue = f"qPoolDynamic{j % 4}"
                    gts.append(gt)
                for j in range(CPT):
                    c = t * CPT + j
                    gt = gts[j]
                    oh = ohpool.tile([P, P], bf16, tag="gatoh")
                    nc.vector.tensor_tensor(
                        out=oh[:], in0=dstslot[:, c:c + 1].to_broadcast([P, P]),
                        in1=iota_t[:], op=OP.is_equal)
                    tp = tppool.tile([P, P], bf16, tag="tp")
                    nc.tensor.transpose(out=tp[:], in_=oh[:], identity=identbf[:])
                    ohT = ohpool.tile([P, P], f32, tag="gatohT")
                    nc.vector.tensor_copy(out=ohT[:], in_=tp[:])
                    adp = tppool.tile([P, HEADS], f32, tag="tp")
                    nc.tensor.matmul(out=adp[:], lhsT=ohT[:],
                                     rhs=a_d_all[:, t * HEADS:(t + 1) * HEADS],
                                     start=True, stop=True)
                    eat = spool.tile([P, HEADS], f32, tag="eat")
                    nc.vector.tensor_add(out=eat[:], in0=gt[:, GD:GD + 8].bitcast(f32),
                                         in1=adp[:])
                    nc.vector.scalar_tensor_tensor(
                        out=eat[:], in0=eat[:], scalar=0.2, in1=eat[:],
                        op0=OP.mult, op1=OP.max)
                    ex = spool.tile([P, HEADS], f32, tag="ex")
                    nc.scalar.activation(out=ex[:], in_=eat[:], func=AF.Exp)
                    sc = wpool.tile([P, 516], bf16, tag="sc")
                    for h in range(HEADS):
                        pos = h * P + (2 if h >= 2 else 0)
                        nc.vector.tensor_scalar_mul(
                            out=sc[:, pos:pos + P], in0=gt[:, h * P:(h + 1) * P],
                            scalar1=ex[:, h:h + 1])
                    nc.vector.tensor_copy(out=sc[:, 256:258], in_=ex[:, 0:2])
                    nc.vector.tensor_copy(out=sc[:, 514:516], in_=ex[:, 2:4])
                    nc.tensor.matmul(out=U1[:], lhsT=oh[:], rhs=sc[:, 0:258],
                                     start=(j == 0), stop=(j == CPT - 1))
                    nc.tensor.matmul(out=U2[:], lhsT=oh[:], rhs=sc[:, 258:516],
                                     start=(j == 0), stop=(j == CPT - 1))
                rden = spool.tile([P, HEADS], f32, tag="rden")
                nc.vector.reciprocal(out=rden[:, 0:2], in_=U1[:, 256:258])
                nc.vector.reciprocal(out=rden[:, 2:4], in_=U2[:, 256:258])
                gat = wpool.tile([P, GD], f32, tag="gat")
                for h in range(HEADS):
                    Ub = U1 if h < 2 else U2
                    pos = (h % 2) * P
                    nc.vector.tensor_scalar_mul(
                        out=gat[:, h * P:(h + 1) * P], in0=Ub[:, pos:pos + P],
                        scalar1=rden[:, h:h + 1])
                ao = pspool.tile([P, P], f32, tag="mm")
                for k in range(4):
                    tp = tppool.tile([P, P], f32, tag="tp")
                    nc.tensor.transpose(out=tp[:], in_=gat[:, k * P:(k + 1) * P],
                                        identity=ident_t[:])
                    aT = wpool.tile([P, P], f32, tag="aT")
                    nc.vector.tensor_copy(out=aT[:], in_=tp[:])
                    nc.tensor.matmul(out=ao[:], lhsT=watt_t[:, k, :], rhs=aT[:],
                                     start=(k == 0), stop=(k == 3))
                nc.scalar.activation(out=aoutT[:, t * P:(t + 1) * P], in_=ao[:],
                                     func=AF.Relu, bias=batt_t[:], scale=1.0)

            # ---------- P8: output projection ----------
            for t in range(TPC):
                po = pspool.tile([P, OUT_DIM], f32, tag="mm")
                nc.tensor.matmul(out=po[:], lhsT=aoutT[:, t * P:(t + 1) * P],
                                 rhs=wout_t[:], start=True, stop=True)
                osb = wpool.tile([P, OUT_DIM], f32, tag="osb")
                nc.vector.tensor_add(out=osb[:], in0=po[:], in1=bout_t[:])
                nc.sync.dma_start(out=out_c[t * P:(t + 1) * P, :], in_=osb[:])

    nc.finalize()
    return nc


_CACHE = {}


def kernel(_trace=False, **inputs):
    from concourse import bass_utils

    ei = np.asarray(inputs["edge_index"])
    src_idx, dst_slot, dinv_new, perm = _preprocess(ei)

    x = np.asarray(inputs["x"], dtype=np.float32)
    xP = np.zeros((N_PAD, IN_DIM), np.float32)
    xP[perm[:N]] = x

    # host weight prep
    g = lambda k: np.asarray(inputs[k], dtype=np.float32)
    Wgat = g("Wgat")
    Wg3 = Wgat.reshape(HID, HEADS, HID)
    Vs = np.einsum("khc,hc->kh", Wg3, g("att_src")).astype(np.float32)
    Vd = np.einsum("khc,hc->kh", Wg3, g("att_dst")).astype(np.float32)
    batt_f = (g("bgat") @ g("Watt_out") + g("batt_out")).astype(np.float32)

    bc = lambda v, w: np.tile(np.asarray(v, np.float32)[None, :w], (P, 1))
    pp = lambda v: np.tile(np.float32(v).reshape(-1, 1), (P, 1))[:P].astype(np.float32)

    common = {
        "idxsrc": None, "dstslot": None, "dinv_t": None,  # per-core below
        "iota_f": np.tile(np.arange(P, dtype=np.float32)[None, :], (P, 1)),
        "ident": np.eye(P, dtype=np.float32),
        "Win": g("Win"),
        "bin_pp": np.tile(g("bin_")[:, None], (1, 1)).astype(np.float32),
        "Wg1": g("Wg1"), "Wg2": g("Wg2"),
        "bg1_bc": bc(g("bg1"), HID), "bg2_bc": bc(g("bg2"), HID),
        "g1g_bc": bc(g("g1_gamma"), HID), "g1b_bc": bc(g("g1_beta"), HID),
        "g2g_bc": bc(g("g2_gamma"), HID), "g2b_bc": bc(g("g2_beta"), HID),
        "Wgat": Wgat,
        "VsVd": np.concatenate([Vs, Vd], axis=1).astype(np.float32),
        "Watt": g("Watt_out"),
        "batt_pp": batt_f[:, None].astype(np.float32),
        "Wout": g("Wout"),
        "bout_bc": bc(g("bout"), OUT_DIM),
        "eps_pp": np.full((P, 1), 1e-5, np.float32),
    }
    common["bin_pp"] = g("bin_")[:, None].astype(np.float32)

    key = "nc"
    if key not in _CACHE:
        _CACHE[key] = _build_nc()
    nc = _CACHE[key]

    in_maps = []
    for c in range(NCORES):
        m = dict(common)
        m["x_c"] = np.ascontiguousarray(xP[c * NPC:(c + 1) * NPC])
        m["idxsrc"] = np.ascontiguousarray(
            src_idx[c].reshape(CHUNKS, P).T).astype(np.int32)
        m["dstslot"] = np.ascontiguousarray(
            dst_slot[c].reshape(CHUNKS, P).T).astype(np.float32)
        m["dinv_t"] = np.ascontiguousarray(
            dinv_new[c * NPC:(c + 1) * NPC].reshape(TPC, P).T).astype(np.float32)
        in_maps.append(m)

    res = bass_utils.run_bass_kernel_spmd(
        nc, in_maps, core_ids=list(range(NCORES)), trace=_trace)

    outP = np.concatenate([res.results[c]["out_c"] for c in range(NCORES)], axis=0)
    out = outP[perm[:N]]
    if _trace:
        kernel._last_exec_ns = res.exec_time_ns
    return out.astype(np.float32)



# revision 5
# speedup vs baseline: 1.4161x; 1.4161x over previous
"""8-core Trainium2 Bass kernel for nn_MetabolicGNN (GCN x2 + GAT + MLP).

Strategy: nodes permuted into 392 degree-balanced tiles of 128 (49 tiles/core);
node tables (bf16) built per layer as core shards and AllGathered. Edges
grouped by dst tile; per tile, src rows are pulled with dma_gather (int16
indices, <=1024 per instruction). The node table is split in two halves so
indices fit int16; a two-pass balancer bounds each tile's low-half and
high-half edge counts so the per-tile gather plan (cA chunks of lows, cB of
highs, 8-chunk gathers) is uniform across cores. Per 128-edge chunk a one-hot
[edge x dst] matrix built in one tensor_scalar(is_equal) op feeds a PE matmul
accumulating into PSUM. GCN normalization folded (dinv[src] pre-scaled rows,
dinv[dst] output scale). GAT: table rows carry [4x(128 feats|1.0) | a_s as
f32-in-bf16 | pad to 640]; per-edge a_d gathered from a core-local padded
DRAM table by dst id; leaky/exp batched per tile; attention weights fused
into the one-hot via one scalar_tensor_tensor per chunk so each head's PSUM
matmul [128x129] accumulates numerator and softmax denominator together.
"""
import sys

sys.path.insert(0, "/opt/trn_rl_repo")

import numpy as np

N = 50000
E = 800000
IN_DIM, HID, OUT_DIM, HEADS = 256, 128, 64, 4
NCORES = 8
P = 128
TPC = 49                    # tiles per core
NT = NCORES * TPC           # 392 tiles
N_PAD = NT * P              # 50176
NPC = TPC * P               # 6272 nodes per core
SPLIT = N_PAD // 2          # 25088 (table half size, int16-safe)
GD = HEADS * HID            # 512
GROW = 640                  # GAT row: 4x129 + 8 (a_s f32x4) + pad -> %256B
AS_OFF = HEADS * 129        # 516
ADW = 128                   # a_d table row width (bf16) -> 256B rows
MAXI = 1024                 # dma_gather per-instruction index cap


def _side_groups(c):
    """Split c chunks into gather groups of <=8 chunks (1024 idxs)."""
    out, ch0 = [], 0
    while c > 0:
        n = min(8, c)
        out.append((ch0, n))
        ch0 += n
        c -= n
    return out


def _wrap16(stream):
    """[L] int array (L%16==0) -> [128, L//16] int16, 16-wrapped, x8 replicated."""
    w = stream.reshape(-1, 16).T
    return np.tile(w, (8, 1)).astype(np.int16)


def _preprocess(edge_index):
    import heapq
    src = edge_index[0].astype(np.int64)
    dst = edge_index[1].astype(np.int64)
    loop = np.arange(N, dtype=np.int64)
    srcA = np.concatenate([src, loop])
    dstA = np.concatenate([dst, loop])
    deg = np.bincount(dstA, minlength=N).astype(np.int64)
    dinv = (1.0 / np.sqrt(deg)).astype(np.float32)

    # ---- pass 1: degree-balanced tiles (defines node halves) ----
    order = np.argsort(-deg, kind="stable")
    fill = np.zeros(NT, np.int64)
    load = np.zeros(NT, np.int64)
    t1 = np.full(N, -1, np.int64)
    heap = [(0, i) for i in range(NT)]
    heapq.heapify(heap)
    for n in order:
        while True:
            ld, i = heapq.heappop(heap)
            if fill[i] < P:
                break
        t1[n] = i
        fill[i] += 1
        load[i] += deg[n]
        if fill[i] < P:
            heapq.heappush(heap, (load[i], i))
    half = (t1 >= NT // 2).astype(np.int8)

    # per-node (low,high) in-edge profile under pass-1 halves
    e_half = half[srcA % N] if False else half[np.minimum(srcA, N - 1)]
    # (srcA < N always: loops and edges index real nodes)
    e_half = half[srcA]
    dl = np.bincount(dstA, weights=(e_half == 0).astype(np.float64),
                     minlength=N).astype(np.int64)
    dh = np.bincount(dstA, weights=(e_half == 1).astype(np.float64),
                     minlength=N).astype(np.int64)

    # ---- pass 2: per-half reassignment balancing max(low,high) ----
    node_tile = np.full(N_PAD, -1, np.int64)
    node_slot = np.full(N_PAD, -1, np.int64)
    for hf in (0, 1):
        tiles = list(range(0, NT // 2)) if hf == 0 else list(range(NT // 2, NT))
        nodes = np.where(half == hf)[0]
        key = dl[nodes] + dh[nodes]
        nodes = nodes[np.argsort(-key, kind="stable")]
        nt = len(tiles)
        fill2 = np.zeros(nt, np.int64)
        llo = np.zeros(nt, np.int64)
        lhi = np.zeros(nt, np.int64)
        heap = [(0, i) for i in range(nt)]
        heapq.heapify(heap)
        for n in nodes:
            while True:
                _, i = heapq.heappop(heap)
                if fill2[i] < P:
                    break
            node_tile[n] = tiles[i]
            node_slot[n] = fill2[i]
            fill2[i] += 1
            llo[i] += dl[n]
            lhi[i] += dh[n]
            if fill2[i] < P:
                heapq.heappush(heap, (max(llo[i], lhi[i]), i))

    # pad node ids fill all remaining slots
    used = np.zeros((NT, P), bool)
    for n in range(N):
        used[node_tile[n], node_slot[n]] = True
    free_all = [(t, s) for t in range(NT) for s in range(P) if not used[t, s]]
    assert len(free_all) == N_PAD - N
    for pn, (t, s) in zip(range(N, N_PAD), free_all):
        node_tile[pn] = t
        node_slot[pn] = s

    perm = node_tile * P + node_slot            # old id -> new id

    # ---- edge lists per tile, lows-first ----
    e_tile = node_tile[dstA]
    e_slot = node_slot[dstA]
    e_srcnew = perm[srcA]
    e_low = (e_srcnew < SPLIT)
    eo = np.lexsort((~e_low, e_tile))   # by tile, lows first
    e_tile, e_slot, e_srcnew, e_low = (
        e_tile[eo], e_slot[eo], e_srcnew[eo], e_low[eo])
    starts = np.searchsorted(e_tile, np.arange(NT))
    ends = np.searchsorted(e_tile, np.arange(NT) + 1)

    nlow = np.zeros(NT, np.int64)
    nhigh = np.zeros(NT, np.int64)
    for t in range(NT):
        s, e = starts[t], ends[t]
        nlow[t] = int(e_low[s:e].sum())
        nhigh[t] = (e - s) - nlow[t]
    cA = int(-(-nlow.max() // P))
    cB = int(-(-nhigh.max() // P))
    CPT = cA + cB

    epc = CPT * P
    dst_slot = np.full((NCORES, TPC * epc), -1.0, dtype=np.float32)
    dst_loc = np.zeros((NCORES, TPC * epc), dtype=np.int64)
    sidx = np.zeros((NCORES, TPC * epc), dtype=np.int64)  # in-half row idx
    for t in range(NT):
        c, tl = divmod(t, TPC)
        s, e = starts[t], ends[t]
        nl = int(nlow[t])
        nh = (e - s) - nl
        base = tl * epc
        sidx[c, base:base + nl] = e_srcnew[s:s + nl]
        sidx[c, base + cA * P:base + cA * P + nh] = \
            e_srcnew[s + nl:e] - SPLIT
        dst_slot[c, base:base + nl] = e_slot[s:s + nl]
        dst_slot[c, base + cA * P:base + cA * P + nh] = e_slot[s + nl:e]
        dst_loc[c, base:base + nl] = tl * P + e_slot[s:s + nl]
        dst_loc[c, base + cA * P:base + cA * P + nh] = \
            tl * P + e_slot[s + nl:e]

    # ---- wrapped int16 streams ----
    groups = [(0, ch0, nch) for ch0, nch in _side_groups(cA)] + \
             [(1, cA + ch0, nch) for ch0, nch in _side_groups(cB)]
    dgroups = _side_groups(CPT)
    scols = epc // 16                   # idx cols per tile
    sidx16 = np.zeros((NCORES, 128, TPC * scols), np.int16)
    didx16 = np.zeros((NCORES, 128, TPC * scols), np.int16)
    for c in range(NCORES):
        for tl in range(TPC):
            base = tl * epc
            col0 = tl * scols
            sidx16[c][:, col0:col0 + scols] = _wrap16(sidx[c, base:base + epc])
            didx16[c][:, col0:col0 + scols] = _wrap16(
                dst_loc[c, base:base + epc])

    dinv_new = np.ones(N_PAD, dtype=np.float32)
    dinv_new[perm[:N]] = dinv
    plan = {"cA": cA, "cB": cB, "CPT": CPT, "groups": groups,
            "dgroups": dgroups}
    return sidx16, didx16, dst_slot, dinv_new, perm, plan


def _build_nc(plan):
    import concourse.bass as bass
    import concourse.bacc as bacc
    import concourse.tile as tile
    from concourse import mybir

    f32 = mybir.dt.float32
    bf16 = mybir.dt.bfloat16
    i16 = mybir.dt.int16
    AF = mybir.ActivationFunctionType
    OP = mybir.AluOpType

    cA, cB, CPT = plan["cA"], plan["cB"], plan["CPT"]
    groups, dgroups = plan["groups"], plan["dgroups"]
    CHUNKS = TPC * CPT
    SCOLS = CPT * P // 16

    nc = bacc.Bacc(trn_type="TRN2", target_bir_lowering=False, num_devices=NCORES,
                   dynamic_dma_scratch_size=32768, num_swdge_queues=4)

    # ---- I/O ----
    x_c = nc.dram_tensor("x_c", [NPC, IN_DIM], f32, kind="ExternalInput")
    sidx_d = nc.dram_tensor("sidx16", [P, TPC * SCOLS], i16, kind="ExternalInput")
    didx_d = nc.dram_tensor("didx16", [P, TPC * SCOLS], i16, kind="ExternalInput")
    dstslot_d = nc.dram_tensor("dstslot", [P, CHUNKS], f32, kind="ExternalInput")
    dinv_d = nc.dram_tensor("dinv_t", [P, TPC], f32, kind="ExternalInput")
    iota_d = nc.dram_tensor("iota_f", [P, P], f32, kind="ExternalInput")
    ident_d = nc.dram_tensor("ident", [P, P], f32, kind="ExternalInput")
    win_d = nc.dram_tensor("Win", [IN_DIM, HID], f32, kind="ExternalInput")
    bin_d = nc.dram_tensor("bin_pp", [P, 1], f32, kind="ExternalInput")
    wg1_d = nc.dram_tensor("Wg1", [HID, HID], f32, kind="ExternalInput")
    wg2_d = nc.dram_tensor("Wg2", [HID, HID], f32, kind="ExternalInput")
    bg1_d = nc.dram_tensor("bg1_bc", [P, HID], f32, kind="ExternalInput")
    bg2_d = nc.dram_tensor("bg2_bc", [P, HID], f32, kind="ExternalInput")
    g1g_d = nc.dram_tensor("g1g_bc", [P, HID], f32, kind="ExternalInput")
    g1b_d = nc.dram_tensor("g1b_bc", [P, HID], f32, kind="ExternalInput")
    g2g_d = nc.dram_tensor("g2g_bc", [P, HID], f32, kind="ExternalInput")
    g2b_d = nc.dram_tensor("g2b_bc", [P, HID], f32, kind="ExternalInput")
    wgat_d = nc.dram_tensor("Wgat", [HID, GD], f32, kind="ExternalInput")
    vsvd_d = nc.dram_tensor("VsVd", [HID, 2 * HEADS], f32, kind="ExternalInput")
    watt_d = nc.dram_tensor("Watt", [GD, HID], f32, kind="ExternalInput")
    batt_d = nc.dram_tensor("batt_pp", [P, 1], f32, kind="ExternalInput")
    wout_d = nc.dram_tensor("Wout", [HID, OUT_DIM], f32, kind="ExternalInput")
    bout_d = nc.dram_tensor("bout_bc", [P, OUT_DIM], f32, kind="ExternalInput")
    eps_d = nc.dram_tensor("eps_pp", [P, 1], f32, kind="ExternalInput")
    out_c = nc.dram_tensor("out_c", [NPC, OUT_DIM], f32, kind="ExternalOutput")

    # ---- internal DRAM ----
    ag_in1 = nc.dram_tensor("ag_in1", [NPC, HID], bf16, kind="Internal")
    tb1 = nc.dram_tensor("tb1", [N_PAD, HID], bf16, kind="Internal", addr_space="Shared")
    ag_in2 = nc.dram_tensor("ag_in2", [NPC, HID], bf16, kind="Internal")
    tb2 = nc.dram_tensor("tb2", [N_PAD, HID], bf16, kind="Internal", addr_space="Shared")
    ag_in3 = nc.dram_tensor("ag_in3", [NPC, GROW], bf16, kind="Internal")
    tb3 = nc.dram_tensor("tb3", [N_PAD, GROW], bf16, kind="Internal", addr_space="Shared")
    ad_d = nc.dram_tensor("ad_d", [NPC, ADW], bf16, kind="Internal")

    rg = [list(range(NCORES))]

    with tile.TileContext(nc) as tc:
        with (
            tc.tile_pool(name="const", bufs=1) as cpool,
            tc.tile_pool(name="big", bufs=1) as bigpool,
            tc.tile_pool(name="gath", bufs=2) as gpool,
            tc.tile_pool(name="adg", bufs=2) as adpool,
            tc.tile_pool(name="oh", bufs=8) as ohpool,
            tc.tile_pool(name="work", bufs=4) as wpool,
            tc.tile_pool(name="small", bufs=10) as spool,
            tc.tile_pool(name="ps", bufs=2, space="PSUM") as pspool,
            tc.tile_pool(name="pst", bufs=2, space="PSUM") as tppool,
        ):
            # ---------- constants ----------
            def cload(dram, shape, dtype=f32):
                t = cpool.tile(shape, dtype, tag="c_" + dram.name)
                nc.sync.dma_start(out=t[:], in_=dram[:])
                return t

            def tobf(src_t, shape, tag):
                t = cpool.tile(shape, bf16, tag=tag)
                nc.vector.tensor_copy(out=t[:], in_=src_t[:])
                return t

            iota_t = cload(iota_d, [P, P])
            ident_t = cload(ident_d, [P, P])
            win_t = cpool.tile([P, IN_DIM // P, HID], f32, tag='c_Win')
            nc.sync.dma_start(out=win_t[:],
                              in_=win_d[:].rearrange("(h p) c -> p h c", p=P))
            bin_t = cload(bin_d, [P, 1])
            wg1_t = cload(wg1_d, [P, HID])
            wg2_t = cload(wg2_d, [P, HID])
            bg1_t = cload(bg1_d, [P, HID])
            bg2_t = cload(bg2_d, [P, HID])
            g1g_t = cload(g1g_d, [P, HID])
            g1b_t = cload(g1b_d, [P, HID])
            g2g_t = cload(g2g_d, [P, HID])
            g2b_t = cload(g2b_d, [P, HID])
            wgat_t = cload(wgat_d, [P, GD])
            vsvd_t = cload(vsvd_d, [P, 2 * HEADS])
            watt_t = cpool.tile([P, GD // P, HID], f32, tag='c_Watt')
            nc.sync.dma_start(out=watt_t[:],
                              in_=watt_d[:].rearrange("(k p) c -> p k c", p=P))
            batt_t = cload(batt_d, [P, 1])
            wout_t = cload(wout_d, [P, OUT_DIM])
            bout_t = cload(bout_d, [P, OUT_DIM])
            eps_t = cload(eps_d, [P, 1])
            dinv_t = cload(dinv_d, [P, TPC])
            sidx = cload(sidx_d, [P, TPC * SCOLS], i16)
            didx = cload(didx_d, [P, TPC * SCOLS], i16)
            dstslot = cload(dstslot_d, [P, CHUNKS])

            identbf = tobf(ident_t, [P, P], "c_identbf")
            iota_bf = tobf(iota_t, [P, P], "c_iotabf")
            iota4 = cpool.tile([P, HEADS, P], bf16, tag="c_iota4")
            for h in range(HEADS):
                nc.vector.tensor_copy(out=iota4[:, h, :], in_=iota_t[:])
            win_bf = tobf(win_t, [P, IN_DIM // P, HID], "c_winbf")
            wg1_bf = tobf(wg1_t, [P, HID], "c_wg1bf")
            wg2_bf = tobf(wg2_t, [P, HID], "c_wg2bf")
            wgat_bf = tobf(wgat_t, [P, GD], "c_wgatbf")
            vsvd_bf = tobf(vsvd_t, [P, 2 * HEADS], "c_vsvdbf")
            watt_bf = tobf(watt_t, [P, GD // P, HID], "c_wattbf")
            wout_bf = tobf(wout_t, [P, OUT_DIM], "c_woutbf")

            h0T = bigpool.tile([P, NPC], bf16, tag="h0T")

            # ---------- P1: h0T = relu(Win.T @ x.T + bin), feature-major ----------
            for ch in range(13):
                n0 = ch * 512
                nn = min(512, NPC - n0)
                nsub = nn // P
                xT = wpool.tile([P, 2, 512], bf16, tag="xT")
                for s in range(nsub):
                    xt = wpool.tile([P, IN_DIM], f32, tag="xload")
                    nc.sync.dma_start(out=xt[:], in_=x_c[n0 + s * P:n0 + (s + 1) * P, :])
                    for h in range(2):
                        tp = tppool.tile([P, P], f32, tag="tp")
                        nc.tensor.transpose(out=tp[:], in_=xt[:, h * P:(h + 1) * P],
                                            identity=ident_t[:])
                        nc.vector.tensor_copy(out=xT[:, h, s * P:(s + 1) * P], in_=tp[:])
                hp = pspool.tile([P, 512], f32, tag="mm")
                for h in range(2):
                    nc.tensor.matmul(out=hp[:, :nn], lhsT=win_bf[:, h, :], rhs=xT[:, h, :nn],
                                     start=(h == 0), stop=(h == 1))
                nc.scalar.activation(out=h0T[:, n0:n0 + nn], in_=hp[:, :nn],
                                     func=AF.Relu, bias=bin_t[:], scale=1.0)

            # ---------- helper: xw table build + AG ----------
            def build_table(srcT, w_t, ag_in, tb, scale_dinv):
                for t in range(TPC):
                    ps = pspool.tile([P, HID], f32, tag="mm")
                    nc.tensor.matmul(out=ps[:], lhsT=srcT[:, t * P:(t + 1) * P],
                                     rhs=w_t[:], start=True, stop=True)
                    sb = wpool.tile([P, HID], bf16, tag="xwsb")
                    if scale_dinv:
                        nc.vector.tensor_scalar_mul(out=sb[:], in0=ps[:],
                                                    scalar1=dinv_t[:, t:t + 1])
                    else:
                        nc.vector.tensor_copy(out=sb[:], in_=ps[:])
                    nc.sync.dma_start(out=ag_in[t * P:(t + 1) * P, :], in_=sb[:])
                nc.gpsimd.collective_compute(
                    "AllGather", OP.bypass, ins=[ag_in[:]], outs=[tb[:]],
                    replica_groups=rg)

            # ---------- helper: gathers for one tile from a split table ----------
            def tile_gathers(gt, tb, t, width, qbase):
                for gi, (hf, ch0, nch) in enumerate(groups):
                    ni = nch * P
                    col0 = t * SCOLS + ch0 * 8
                    view = tb[0:SPLIT, :] if hf == 0 else tb[SPLIT:N_PAD, :]
                    nc.gpsimd.dma_gather(
                        gt[:, ch0:ch0 + nch, :], view,
                        sidx[:, col0:col0 + nch * 8], ni, ni, width,
                        queue_num=(qbase + gi) % 4)

            # ---------- helper: layernorm(+relu) on node-major tile ----------
            def ln_relu(dst, src, gam, bet):
                st = spool.tile([P, 6], f32, tag="lnst")
                nc.vector.bn_stats(out=st[:], in_=src[:])
                mv = spool.tile([P, 2], f32, tag="lnmv")
                nc.vector.bn_aggr(out=mv[:], in_=st[:])
                rstd = spool.tile([P, 1], f32, tag="lnrs")
                nc.scalar.activation(out=rstd[:], in_=mv[:, 1:2], func=AF.Sqrt,
                                     bias=eps_t[:], scale=1.0)
                nc.vector.reciprocal(out=rstd[:], in_=rstd[:])
                nc.vector.tensor_scalar(out=src[:], in0=src[:], scalar1=mv[:, 0:1],
                                        scalar2=rstd[:], op0=OP.subtract, op1=OP.mult)
                nc.vector.tensor_mul(out=src[:], in0=src[:], in1=gam[:])
                nc.vector.tensor_add(out=src[:], in0=src[:], in1=bet[:])
                nc.scalar.activation(out=dst[:], in_=src[:], func=AF.Relu)

            # ---------- helper: GCN aggregation pass ----------
            def gcn_pass(tb, bg_t, gam, bet, outT, resT):
                for t in range(TPC):
                    gt = gpool.tile([P, CPT, HID], bf16, tag="gcng")
                    tile_gathers(gt, tb, t, HID, qbase=t)
                    U = pspool.tile([P, HID], f32, tag="U1")
                    for j in range(CPT):
                        c = t * CPT + j
                        oh = ohpool.tile([P, P], bf16, tag="gcnoh")
                        nc.vector.tensor_scalar(out=oh[:], in0=iota_bf[:],
                                                scalar1=dstslot[:, c:c + 1],
                                                scalar2=None, op0=OP.is_equal)
                        nc.tensor.matmul(out=U[:], lhsT=oh[:], rhs=gt[:, j, :],
                                         start=(j == 0), stop=(j == CPT - 1))
                    pre = wpool.tile([P, HID], f32, tag="gcnpre")
                    nc.vector.scalar_tensor_tensor(
                        out=pre[:], in0=U[:], scalar=dinv_t[:, t:t + 1], in1=bg_t[:],
                        op0=OP.mult, op1=OP.add)
                    nm = wpool.tile([P, HID], bf16, tag="gcnnm")
                    ln_relu(nm, pre, gam, bet)
                    tp = tppool.tile([P, P], bf16, tag="tp")
                    nc.tensor.transpose(out=tp[:], in_=nm[:], identity=identbf[:])
                    if resT is None:
                        nc.vector.tensor_copy(out=outT[:, t * P:(t + 1) * P], in_=tp[:])
                    else:
                        nc.vector.tensor_add(out=outT[:, t * P:(t + 1) * P],
                                             in0=resT[:, t * P:(t + 1) * P], in1=tp[:])

            # ---------- GCN layer 1 ----------
            build_table(h0T, wg1_bf, ag_in1, tb1, True)
            x1T = bigpool.tile([P, NPC], bf16, tag="x1T")
            gcn_pass(tb1, bg1_t, g1g_t, g1b_t, x1T, None)

            # ---------- GCN layer 2 (residual) ----------
            build_table(x1T, wg2_bf, ag_in2, tb2, True)
            x2T = bigpool.tile([P, NPC], bf16, tag="h0T")  # reuse h0T slot
            gcn_pass(tb2, bg2_t, g2g_t, g2b_t, x2T, x1T)

            # ---------- P6: GAT table ----------
            for t in range(TPC):
                ps = pspool.tile([P, GD], f32, tag="mm")
                nc.tensor.matmul(out=ps[:], lhsT=x2T[:, t * P:(t + 1) * P],
                                 rhs=wgat_bf[:], start=True, stop=True)
                ps8 = tppool.tile([P, 2 * HEADS], f32, tag="tp")
                nc.tensor.matmul(out=ps8[:], lhsT=x2T[:, t * P:(t + 1) * P],
                                 rhs=vsvd_bf[:], start=True, stop=True)
                stg = wpool.tile([P, GROW], bf16, tag="stg")
                stg4 = stg[:, 0:HEADS * 129].rearrange("p (h c) -> p h c", h=HEADS)
                nc.vector.tensor_copy(
                    out=stg4[:, :, 0:P],
                    in_=ps[:].rearrange("p (h c) -> p h c", h=HEADS))
                nc.vector.memset(stg4[:, :, P:P + 1], 1.0)
                stgf = stg[:, AS_OFF:AS_OFF + 8].bitcast(f32)
                nc.vector.tensor_copy(out=stgf, in_=ps8[:, 0:HEADS])
                adsb = spool.tile([P, 8], bf16, tag="adsb")
                nc.vector.tensor_copy(out=adsb[:].bitcast(f32),
                                      in_=ps8[:, HEADS:2 * HEADS])
                nc.sync.dma_start(out=ad_d[t * P:(t + 1) * P, 0:8], in_=adsb[:])
                nc.sync.dma_start(out=ag_in3[t * P:(t + 1) * P, :], in_=stg[:])
            nc.gpsimd.collective_compute(
                "AllGather", OP.bypass, ins=[ag_in3[:]], outs=[tb3[:]],
                replica_groups=rg)

            # ---------- P7: GAT aggregation + att_out ----------
            aoutT = bigpool.tile([P, NPC], bf16, tag="x1T")  # reuse x1T slot
            for t in range(TPC):
                gt = gpool.tile([P, CPT, GROW], bf16, tag="gatg")
                tile_gathers(gt, tb3, t, GROW, qbase=t)
                ad = adpool.tile([P, CPT, ADW], bf16, tag="adg")
                for gi, (ch0, nch) in enumerate(dgroups):
                    ni = nch * P
                    col0 = t * SCOLS + ch0 * 8
                    nc.gpsimd.dma_gather(
                        ad[:, ch0:ch0 + nch, :], ad_d[:],
                        didx[:, col0:col0 + nch * 8], ni, ni, ADW,
                        queue_num=(t + gi + 2) % 4)

                # batched eat = leaky_relu(a_s + a_d); ex = exp(eat)
                ea = spool.tile([P, CPT, HEADS], f32, tag="ea")
                nc.vector.tensor_tensor(
                    out=ea[:], in0=gt[:, :, AS_OFF:AS_OFF + 8].bitcast(f32),
                    in1=ad[:, :, 0:8].bitcast(f32), op=OP.add)
                eaf = ea[:].rearrange("p a b -> p (a b)")
                nc.vector.scalar_tensor_tensor(
                    out=eaf, in0=eaf, scalar=0.2, in1=eaf,
                    op0=OP.mult, op1=OP.max)
                ex = spool.tile([P, CPT, HEADS], f32, tag="ex")
                nc.scalar.activation(out=ex[:].rearrange("p a b -> p (a b)"),
                                     in_=eaf, func=AF.Exp)

                U12 = pspool.tile([P, 258], f32, tag="U1")
                U34 = pspool.tile([P, 258], f32, tag="U2")
                for j in range(CPT):
                    c = t * CPT + j
                    ohw = ohpool.tile([P, HEADS, P], bf16, tag="gatohw")
                    nc.vector.scalar_tensor_tensor(
                        out=ohw[:], in0=iota4[:],
                        scalar=dstslot[:, c:c + 1],
                        in1=ex[:, j, :].to_broadcast([P, HEADS, P]),
                        op0=OP.is_equal, op1=OP.mult)
                    for h in range(HEADS):
                        Ub = U12 if h < 2 else U34
                        pos = (h % 2) * 129
                        nc.tensor.matmul(
                            out=Ub[:, pos:pos + 129], lhsT=ohw[:, h, :],
                            rhs=gt[:, j, h * 129:(h + 1) * 129],
                            start=(j == 0), stop=(j == CPT - 1))

                rden = spool.tile([P, HEADS], f32, tag="rden")
                nc.vector.reciprocal(
                    out=rden[:, 0:2].rearrange("p (a b) -> p a b", b=1),
                    in_=U12[:].rearrange("p (a b) -> p a b", a=2)[:, :, 128:129])
                nc.vector.reciprocal(
                    out=rden[:, 2:4].rearrange("p (a b) -> p a b", b=1),
                    in_=U34[:].rearrange("p (a b) -> p a b", a=2)[:, :, 128:129])
                gat = wpool.tile([P, GD], bf16, tag="gat")
                for h in range(HEADS):
                    Ub = U12 if h < 2 else U34
                    pos = (h % 2) * 129
                    nc.vector.tensor_scalar_mul(
                        out=gat[:, h * P:(h + 1) * P], in0=Ub[:, pos:pos + P],
                        scalar1=rden[:, h:h + 1])
                ao = pspool.tile([P, P], f32, tag="mm")
                for k in range(4):
                    tp = tppool.tile([P, P], bf16, tag="tp")
                    nc.tensor.transpose(out=tp[:], in_=gat[:, k * P:(k + 1) * P],
                                        identity=identbf[:])
                    aT = wpool.tile([P, P], bf16, tag="aT")
                    nc.vector.tensor_copy(out=aT[:], in_=tp[:])
                    nc.tensor.matmul(out=ao[:], lhsT=watt_bf[:, k, :], rhs=aT[:],
                                     start=(k == 0), stop=(k == 3))
                nc.scalar.activation(out=aoutT[:, t * P:(t + 1) * P], in_=ao[:],
                                     func=AF.Relu, bias=batt_t[:], scale=1.0)

            # ---------- P8: output projection ----------
            for t in range(TPC):
                po = pspool.tile([P, OUT_DIM], f32, tag="mm")
                nc.tensor.matmul(out=po[:], lhsT=aoutT[:, t * P:(t + 1) * P],
                                 rhs=wout_bf[:], start=True, stop=True)
                osb = wpool.tile([P, OUT_DIM], f32, tag="osb")
                nc.vector.tensor_add(out=osb[:], in0=po[:], in1=bout_t[:])
                nc.sync.dma_start(out=out_c[t * P:(t + 1) * P, :], in_=osb[:])

    nc.finalize()
    return nc


_CACHE = {}


def kernel(_trace=False, **inputs):
    from concourse import bass_utils

    ei = np.asarray(inputs["edge_index"])
    sidx16, didx16, dst_slot, dinv_new, perm, plan = _preprocess(ei)
    CPT = plan["CPT"]
    CHUNKS = TPC * CPT

    x = np.asarray(inputs["x"], dtype=np.float32)
    xP = np.zeros((N_PAD, IN_DIM), np.float32)
    xP[perm[:N]] = x

    g = lambda k: np.asarray(inputs[k], dtype=np.float32)
    Wgat = g("Wgat")
    Wg3 = Wgat.reshape(HID, HEADS, HID)
    Vs = np.einsum("khc,hc->kh", Wg3, g("att_src")).astype(np.float32)
    Vd = np.einsum("khc,hc->kh", Wg3, g("att_dst")).astype(np.float32)
    batt_f = (g("bgat") @ g("Watt_out") + g("batt_out")).astype(np.float32)

    bc = lambda v, w: np.tile(np.asarray(v, np.float32)[None, :w], (P, 1))

    common = {
        "iota_f": np.tile(np.arange(P, dtype=np.float32)[None, :], (P, 1)),
        "ident": np.eye(P, dtype=np.float32),
        "Win": g("Win"),
        "bin_pp": g("bin_")[:, None].astype(np.float32),
        "Wg1": g("Wg1"), "Wg2": g("Wg2"),
        "bg1_bc": bc(g("bg1"), HID), "bg2_bc": bc(g("bg2"), HID),
        "g1g_bc": bc(g("g1_gamma"), HID), "g1b_bc": bc(g("g1_beta"), HID),
        "g2g_bc": bc(g("g2_gamma"), HID), "g2b_bc": bc(g("g2_beta"), HID),
        "Wgat": Wgat,
        "VsVd": np.concatenate([Vs, Vd], axis=1).astype(np.float32),
        "Watt": g("Watt_out"),
        "batt_pp": batt_f[:, None].astype(np.float32),
        "Wout": g("Wout"),
        "bout_bc": bc(g("bout"), OUT_DIM),
        "eps_pp": np.full((P, 1), 1e-5, np.float32),
    }

    key = ("nc", plan["cA"], plan["cB"])
    if key not in _CACHE:
        _CACHE[key] = _build_nc(plan)
    nc = _CACHE[key]

    in_maps = []
    for c in range(NCORES):
        m = dict(common)
        m["x_c"] = np.ascontiguousarray(xP[c * NPC:(c + 1) * NPC])
        m["sidx16"] = np.ascontiguousarray(sidx16[c])
        m["didx16"] = np.ascontiguousarray(didx16[c])
        m["dstslot"] = np.ascontiguousarray(
            dst_slot[c].reshape(CHUNKS, P).T).astype(np.float32)
        m["dinv_t"] = np.ascontiguousarray(
            dinv_new[c * NPC:(c + 1) * NPC].reshape(TPC, P).T).astype(np.float32)
        in_maps.append(m)

    res = bass_utils.run_bass_kernel_spmd(
        nc, in_maps, core_ids=list(range(NCORES)), trace=_trace)

    outP = np.concatenate([res.results[c]["out_c"] for c in range(NCORES)], axis=0)
    out = outP[perm[:N]]
    if _trace:
        kernel._last_exec_ns = res.exec_time_ns
    return out.astype(np.float32)


# revision 14
# speedup vs baseline: 1.7881x; 1.2627x over previous
"""8-core Trainium2 Bass kernel for nn_MetabolicGNN (GCN x2 + GAT + MLP).

Strategy: nodes permuted into 392 degree-balanced tiles of 128 (49 tiles/core);
node tables (bf16) built per layer as core shards and AllGathered. Edges
grouped by dst tile; per tile, src rows are pulled with dma_gather (int16
indices, <=1024 per instruction). The node table is split in two halves so
indices fit int16; a two-pass balancer bounds each tile's low-half and
high-half edge counts so the per-tile gather plan (cA chunks of lows, cB of
highs, 8-chunk gathers) is uniform across cores. Per 128-edge chunk a one-hot
[edge x dst] matrix built in one tensor_scalar(is_equal) op feeds a PE matmul
accumulating into PSUM. GCN normalization folded (dinv[src] pre-scaled rows,
dinv[dst] output scale). GAT: table rows carry [4x(128 feats|1.0) | a_s as
f32-in-bf16 | pad to 640]; per-edge a_d gathered from a core-local padded
DRAM table by dst id; leaky/exp batched per tile; attention weights fused
into the one-hot via one scalar_tensor_tensor per chunk so each head's PSUM
matmul [128x129] accumulates numerator and softmax denominator together.
"""
import sys

sys.path.insert(0, "/opt/trn_rl_repo")

import numpy as np

N = 50000
E = 800000
IN_DIM, HID, OUT_DIM, HEADS = 256, 128, 64, 4
NCORES = 8
P = 128
TPC = 49                    # tiles per core
NT = NCORES * TPC           # 392 tiles
N_PAD = NT * P              # 50176
NPC = TPC * P               # 6272 nodes per core
SPLIT = N_PAD // 2          # 25088 (table half size, int16-safe)
GD = HEADS * HID            # 512
GROW = 640                  # GAT row: 4x129 + 8 (a_s f32x4) + pad -> %256B
AS_OFF = HEADS * 129        # 516
ADW = 128                   # a_d table row width (bf16) -> 256B rows
MAXI = 1024                 # dma_gather per-instruction index cap


def _side_groups(c):
    """Split c chunks into gather groups of <=8 chunks (1024 idxs)."""
    out, ch0 = [], 0
    while c > 0:
        n = min(8, c)
        out.append((ch0, n))
        ch0 += n
        c -= n
    return out


def _wrap16(stream):
    """[L] int array (L%16==0) -> [128, L//16] int16, 16-wrapped, x8 replicated."""
    w = stream.reshape(-1, 16).T
    return np.tile(w, (8, 1)).astype(np.int16)


def _preprocess(edge_index):
    import heapq
    src = edge_index[0].astype(np.int64)
    dst = edge_index[1].astype(np.int64)
    loop = np.arange(N, dtype=np.int64)
    srcA = np.concatenate([src, loop])
    dstA = np.concatenate([dst, loop])
    deg = np.bincount(dstA, minlength=N).astype(np.int64)
    dinv = (1.0 / np.sqrt(deg)).astype(np.float32)

    # ---- pass 1: degree-balanced tiles (defines node halves) ----
    order = np.argsort(-deg, kind="stable")
    fill = np.zeros(NT, np.int64)
    load = np.zeros(NT, np.int64)
    t1 = np.full(N, -1, np.int64)
    heap = [(0, i) for i in range(NT)]
    heapq.heapify(heap)
    for n in order:
        while True:
            ld, i = heapq.heappop(heap)
            if fill[i] < P:
                break
        t1[n] = i
        fill[i] += 1
        load[i] += deg[n]
        if fill[i] < P:
            heapq.heappush(heap, (load[i], i))
    half = (t1 >= NT // 2).astype(np.int8)

    # per-node (low,high) in-edge profile under pass-1 halves
    e_half = half[srcA % N] if False else half[np.minimum(srcA, N - 1)]
    # (srcA < N always: loops and edges index real nodes)
    e_half = half[srcA]
    dl = np.bincount(dstA, weights=(e_half == 0).astype(np.float64),
                     minlength=N).astype(np.int64)
    dh = np.bincount(dstA, weights=(e_half == 1).astype(np.float64),
                     minlength=N).astype(np.int64)

    # ---- pass 2: per-half reassignment balancing max(low,high) ----
    node_tile = np.full(N_PAD, -1, np.int64)
    node_slot = np.full(N_PAD, -1, np.int64)
    for hf in (0, 1):
        tiles = list(range(0, NT // 2)) if hf == 0 else list(range(NT // 2, NT))
        nodes = np.where(half == hf)[0]
        key = dl[nodes] + dh[nodes]
        nodes = nodes[np.argsort(-key, kind="stable")]
        nt = len(tiles)
        fill2 = np.zeros(nt, np.int64)
        llo = np.zeros(nt, np.int64)
        lhi = np.zeros(nt, np.int64)
        heap = [(0, i) for i in range(nt)]
        heapq.heapify(heap)
        for n in nodes:
            while True:
                _, i = heapq.heappop(heap)
                if fill2[i] < P:
                    break
            node_tile[n] = tiles[i]
            node_slot[n] = fill2[i]
            fill2[i] += 1
            llo[i] += dl[n]
            lhi[i] += dh[n]
            if fill2[i] < P:
                heapq.heappush(heap, (max(llo[i], lhi[i]), i))

    # pad node ids fill all remaining slots
    used = np.zeros((NT, P), bool)
    for n in range(N):
        used[node_tile[n], node_slot[n]] = True
    free_all = [(t, s) for t in range(NT) for s in range(P) if not used[t, s]]
    assert len(free_all) == N_PAD - N
    for pn, (t, s) in zip(range(N, N_PAD), free_all):
        node_tile[pn] = t
        node_slot[pn] = s

    perm = node_tile * P + node_slot            # old id -> new id

    # ---- edge lists per tile, lows-first ----
    e_tile = node_tile[dstA]
    e_slot = node_slot[dstA]
    e_srcnew = perm[srcA]
    e_low = (e_srcnew < SPLIT)
    eo = np.lexsort((~e_low, e_tile))   # by tile, lows first
    e_tile, e_slot, e_srcnew, e_low = (
        e_tile[eo], e_slot[eo], e_srcnew[eo], e_low[eo])
    starts = np.searchsorted(e_tile, np.arange(NT))
    ends = np.searchsorted(e_tile, np.arange(NT) + 1)

    nlow = np.zeros(NT, np.int64)
    nhigh = np.zeros(NT, np.int64)
    for t in range(NT):
        s, e = starts[t], ends[t]
        nlow[t] = int(e_low[s:e].sum())
        nhigh[t] = (e - s) - nlow[t]
    cA = int(-(-nlow.max() // P))
    cB = int(-(-nhigh.max() // P))
    CPT = cA + cB

    epc = CPT * P
    dst_slot = np.full((NCORES, TPC * epc), -1.0, dtype=np.float32)
    dst_loc = np.full((NCORES, TPC * epc), -1, dtype=np.int64)
    sidx = np.full((NCORES, TPC * epc), -1, dtype=np.int64)  # in-half row idx
    nl_ct = np.zeros((NCORES, TPC), np.int64)
    nh_ct = np.zeros((NCORES, TPC), np.int64)
    for t in range(NT):
        c, tl = divmod(t, TPC)
        s, e = starts[t], ends[t]
        nl = int(nlow[t])
        nh = (e - s) - nl
        nl_ct[c, tl] = nl
        nh_ct[c, tl] = nh
        base = tl * epc
        sidx[c, base:base + nl] = e_srcnew[s:s + nl]
        sidx[c, base + cA * P:base + cA * P + nh] = \
            e_srcnew[s + nl:e] - SPLIT
        dst_slot[c, base:base + nl] = e_slot[s:s + nl]
        dst_slot[c, base + cA * P:base + cA * P + nh] = e_slot[s + nl:e]
        dst_loc[c, base:base + nl] = tl * P + e_slot[s:s + nl]
        dst_loc[c, base + cA * P:base + cA * P + nh] = \
            tl * P + e_slot[s + nl:e]

    # ---- gather groups, per-group valid counts, -1 pads with 0 sentinels ----
    groups = [(0, ch0, nch) for ch0, nch in _side_groups(cA)] + \
             [(1, cA + ch0, nch) for ch0, nch in _side_groups(cB)]
    dgroups = _side_groups(CPT)
    NG = len(groups) + len(dgroups)     # counts per tile (src groups + ad)
    counts = np.zeros((NCORES, TPC * 8), np.int32)
    for c in range(NCORES):
        for tl in range(TPC):
            base = tl * epc
            full = tl < 2   # maiden pool buffers: gather fully (0-pads)
            for gi, (hf, ch0, nch) in enumerate(groups):
                g0 = base + ch0 * P
                cap = nch * P
                nv = int(nl_ct[c, tl]) - ch0 * P if hf == 0 else \
                    int(nh_ct[c, tl]) - (ch0 - cA) * P
                nv = max(0, min(cap, nv))
                if full:
                    sidx[c, g0 + nv:g0 + cap] = 0
                    nv = cap
                elif nv < cap:
                    sidx[c, g0 + nv] = 0      # sentinel keeps count >= 1
                    nv += 1
                counts[c, tl * 8 + gi] = nv
            for gi, (ch0, nch) in enumerate(dgroups):
                g0 = base + ch0 * P
                cap = nch * P
                seg = dst_loc[c, g0:g0 + cap]
                nv = int((seg >= 0).sum())   # valid prefix (pads interspersed?)
                # dst stream pads mirror src layout: valid entries are a
                # prefix within each side; count = last valid + 1
                valid_pos = np.nonzero(seg >= 0)[0]
                nv = int(valid_pos[-1]) + 1 if len(valid_pos) else 0
                if full:
                    seg[seg < 0] = 0
                    nv = cap
                else:
                    pad_pos = np.nonzero(seg < 0)[0]
                    keep = pad_pos[pad_pos < nv]
                    seg[keep] = 0            # interior pads must stay valid
                    if nv < cap:
                        seg[nv] = 0          # sentinel
                        nv += 1
                    nv = max(nv, 1)
                dst_loc[c, g0:g0 + cap] = seg
                counts[c, tl * 8 + len(groups) + gi] = nv

    # ---- wrapped int16 streams ----
    scols = epc // 16                   # idx cols per tile
    sidx16 = np.zeros((NCORES, 128, TPC * scols), np.int16)
    didx16 = np.zeros((NCORES, 128, TPC * scols), np.int16)
    for c in range(NCORES):
        for tl in range(TPC):
            base = tl * epc
            col0 = tl * scols
            sidx16[c][:, col0:col0 + scols] = _wrap16(sidx[c, base:base + epc])
            didx16[c][:, col0:col0 + scols] = _wrap16(
                dst_loc[c, base:base + epc])

    dinv_new = np.ones(N_PAD, dtype=np.float32)
    dinv_new[perm[:N]] = dinv
    plan = {"cA": cA, "cB": cB, "CPT": CPT, "groups": groups,
            "dgroups": dgroups}
    return sidx16, didx16, dst_slot, dinv_new, perm, plan, counts


def _build_nc(plan):
    import concourse.bass as bass
    import concourse.bacc as bacc
    import concourse.tile as tile
    from concourse import mybir

    f32 = mybir.dt.float32
    bf16 = mybir.dt.bfloat16
    i16 = mybir.dt.int16
    AF = mybir.ActivationFunctionType
    OP = mybir.AluOpType

    cA, cB, CPT = plan["cA"], plan["cB"], plan["CPT"]
    groups, dgroups = plan["groups"], plan["dgroups"]
    CHUNKS = TPC * CPT
    SCOLS = CPT * P // 16

    nc = bacc.Bacc(trn_type="TRN2", target_bir_lowering=False, num_devices=NCORES,
                   dynamic_dma_scratch_size=32768, num_swdge_queues=4)

    # ---- I/O ----
    i32 = mybir.dt.int32
    x_c = nc.dram_tensor("x_c", [NPC, IN_DIM], f32, kind="ExternalInput")
    sidx_d = nc.dram_tensor("sidx16", [P, TPC * SCOLS], i16, kind="ExternalInput")
    didx_d = nc.dram_tensor("didx16", [P, TPC * SCOLS], i16, kind="ExternalInput")
    cnt_d = nc.dram_tensor("counts", [P, TPC * 8], i32, kind="ExternalInput")
    dstslot_d = nc.dram_tensor("dstslot", [P, CHUNKS], f32, kind="ExternalInput")
    dinv_d = nc.dram_tensor("dinv_t", [P, TPC], f32, kind="ExternalInput")
    iota_d = nc.dram_tensor("iota_f", [P, P], f32, kind="ExternalInput")
    ident_d = nc.dram_tensor("ident", [P, P], f32, kind="ExternalInput")
    win_d = nc.dram_tensor("Win", [IN_DIM, HID], f32, kind="ExternalInput")
    bin_d = nc.dram_tensor("bin_pp", [P, 1], f32, kind="ExternalInput")
    wg1_d = nc.dram_tensor("Wg1", [HID, HID], f32, kind="ExternalInput")
    wg2_d = nc.dram_tensor("Wg2", [HID, HID], f32, kind="ExternalInput")
    bg1_d = nc.dram_tensor("bg1_bc", [P, HID], f32, kind="ExternalInput")
    bg2_d = nc.dram_tensor("bg2_bc", [P, HID], f32, kind="ExternalInput")
    g1g_d = nc.dram_tensor("g1g_bc", [P, HID], f32, kind="ExternalInput")
    g1b_d = nc.dram_tensor("g1b_bc", [P, HID], f32, kind="ExternalInput")
    g2g_d = nc.dram_tensor("g2g_bc", [P, HID], f32, kind="ExternalInput")
    g2b_d = nc.dram_tensor("g2b_bc", [P, HID], f32, kind="ExternalInput")
    wgat_d = nc.dram_tensor("Wgat", [HID, GD], f32, kind="ExternalInput")
    vsvd_d = nc.dram_tensor("VsVd", [HID, 2 * HEADS], f32, kind="ExternalInput")
    watt_d = nc.dram_tensor("Watt", [GD, HID], f32, kind="ExternalInput")
    batt_d = nc.dram_tensor("batt_pp", [P, 1], f32, kind="ExternalInput")
    wout_d = nc.dram_tensor("Wout", [HID, OUT_DIM], f32, kind="ExternalInput")
    bout_d = nc.dram_tensor("bout_bc", [P, OUT_DIM], f32, kind="ExternalInput")
    eps_d = nc.dram_tensor("eps_pp", [P, 1], f32, kind="ExternalInput")
    out_c = nc.dram_tensor("out_c", [NPC, OUT_DIM], f32, kind="ExternalOutput")

    # ---- internal DRAM ----
    ag_in1 = nc.dram_tensor("ag_in1", [NPC, HID], bf16, kind="Internal")
    tb1 = nc.dram_tensor("tb1", [N_PAD, HID], bf16, kind="Internal", addr_space="Shared")
    ag_in2 = nc.dram_tensor("ag_in2", [NPC, HID], bf16, kind="Internal")
    tb2 = nc.dram_tensor("tb2", [N_PAD, HID], bf16, kind="Internal", addr_space="Shared")
    ag_in3 = nc.dram_tensor("ag_in3", [NPC, GROW], bf16, kind="Internal")
    tb3 = nc.dram_tensor("tb3", [N_PAD, GROW], bf16, kind="Internal", addr_space="Shared")
    ad_d = nc.dram_tensor("ad_d", [NPC, ADW], bf16, kind="Internal")

    rg = [list(range(NCORES))]

    with tile.TileContext(nc) as tc:
        with (
            tc.tile_pool(name="const", bufs=1) as cpool,
            tc.tile_pool(name="big", bufs=1) as bigpool,
            tc.tile_pool(name="gath", bufs=2) as gpool,
            tc.tile_pool(name="adg", bufs=2) as adpool,
            tc.tile_pool(name="oh", bufs=8) as ohpool,
            tc.tile_pool(name="work", bufs=4) as wpool,
            tc.tile_pool(name="small", bufs=10) as spool,
            tc.tile_pool(name="ps", bufs=2, space="PSUM") as pspool,
            tc.tile_pool(name="pst", bufs=2, space="PSUM") as tppool,
        ):
            # ---------- constants ----------
            def cload(dram, shape, dtype=f32):
                t = cpool.tile(shape, dtype, tag="c_" + dram.name)
                nc.sync.dma_start(out=t[:], in_=dram[:])
                return t

            def tobf(src_t, shape, tag):
                t = cpool.tile(shape, bf16, tag=tag)
                nc.vector.tensor_copy(out=t[:], in_=src_t[:])
                return t

            iota_t = cload(iota_d, [P, P])
            ident_t = cload(ident_d, [P, P])
            win_t = cpool.tile([P, IN_DIM // P, HID], f32, tag='c_Win')
            nc.sync.dma_start(out=win_t[:],
                              in_=win_d[:].rearrange("(h p) c -> p h c", p=P))
            bin_t = cload(bin_d, [P, 1])
            wg1_t = cload(wg1_d, [P, HID])
            wg2_t = cload(wg2_d, [P, HID])
            bg1_t = cload(bg1_d, [P, HID])
            bg2_t = cload(bg2_d, [P, HID])
            g1g_t = cload(g1g_d, [P, HID])
            g1b_t = cload(g1b_d, [P, HID])
            g2g_t = cload(g2g_d, [P, HID])
            g2b_t = cload(g2b_d, [P, HID])
            wgat_t = cload(wgat_d, [P, GD])
            vsvd_t = cload(vsvd_d, [P, 2 * HEADS])
            watt_t = cpool.tile([P, GD // P, HID], f32, tag='c_Watt')
            nc.sync.dma_start(out=watt_t[:],
                              in_=watt_d[:].rearrange("(k p) c -> p k c", p=P))
            batt_t = cload(batt_d, [P, 1])
            wout_t = cload(wout_d, [P, OUT_DIM])
            bout_t = cload(bout_d, [P, OUT_DIM])
            eps_t = cload(eps_d, [P, 1])
            dinv_t = cload(dinv_d, [P, TPC])
            sidx = cload(sidx_d, [P, TPC * SCOLS], i16)
            didx = cload(didx_d, [P, TPC * SCOLS], i16)
            cnt = cload(cnt_d, [P, TPC * 8], i32)
            dstslot = cload(dstslot_d, [P, CHUNKS])
            NG = len(groups) + len(dgroups)
            gregs = [nc.alloc_register(mybir.EngineType.Pool, f"gcnt{i}")
                     for i in range(NG)]

            identbf = tobf(ident_t, [P, P], "c_identbf")
            iota_bf = tobf(iota_t, [P, P], "c_iotabf")
            iota4 = cpool.tile([P, HEADS, P], bf16, tag="c_iota4")
            for h in range(HEADS):
                nc.vector.tensor_copy(out=iota4[:, h, :], in_=iota_t[:])
            win_bf = tobf(win_t, [P, IN_DIM // P, HID], "c_winbf")
            wg1_bf = tobf(wg1_t, [P, HID], "c_wg1bf")
            wg2_bf = tobf(wg2_t, [P, HID], "c_wg2bf")
            wgat_bf = tobf(wgat_t, [P, GD], "c_wgatbf")
            vsvd_bf = tobf(vsvd_t, [P, 2 * HEADS], "c_vsvdbf")
            watt_bf = tobf(watt_t, [P, GD // P, HID], "c_wattbf")
            wout_bf = tobf(wout_t, [P, OUT_DIM], "c_woutbf")

            h0T = bigpool.tile([P, NPC], bf16, tag="h0T")

            # ---------- P1: h0T = relu(Win.T @ x.T + bin), feature-major ----------
            for ch in range(13):
                n0 = ch * 512
                nn = min(512, NPC - n0)
                nsub = nn // P
                xT = wpool.tile([P, 2, 512], bf16, tag="xT")
                for s in range(nsub):
                    xt = wpool.tile([P, IN_DIM], f32, tag="xload")
                    nc.sync.dma_start(out=xt[:], in_=x_c[n0 + s * P:n0 + (s + 1) * P, :])
                    for h in range(2):
                        tp = tppool.tile([P, P], f32, tag="tp")
                        nc.tensor.transpose(out=tp[:], in_=xt[:, h * P:(h + 1) * P],
                                            identity=ident_t[:])
                        nc.vector.tensor_copy(out=xT[:, h, s * P:(s + 1) * P], in_=tp[:])
                hp = pspool.tile([P, 512], f32, tag="mm")
                for h in range(2):
                    nc.tensor.matmul(out=hp[:, :nn], lhsT=win_bf[:, h, :], rhs=xT[:, h, :nn],
                                     start=(h == 0), stop=(h == 1))
                nc.scalar.activation(out=h0T[:, n0:n0 + nn], in_=hp[:, :nn],
                                     func=AF.Relu, bias=bin_t[:], scale=1.0)

            # ---------- helper: xw table build + AG ----------
            def build_table(srcT, w_t, ag_in, tb, scale_dinv):
                for t in range(TPC):
                    ps = pspool.tile([P, HID], f32, tag="mm")
                    nc.tensor.matmul(out=ps[:], lhsT=srcT[:, t * P:(t + 1) * P],
                                     rhs=w_t[:], start=True, stop=True)
                    sb = wpool.tile([P, HID], bf16, tag="xwsb")
                    if scale_dinv:
                        nc.vector.tensor_scalar_mul(out=sb[:], in0=ps[:],
                                                    scalar1=dinv_t[:, t:t + 1])
                    else:
                        nc.vector.tensor_copy(out=sb[:], in_=ps[:])
                    nc.sync.dma_start(out=ag_in[t * P:(t + 1) * P, :], in_=sb[:])
                nc.gpsimd.collective_compute(
                    "AllGather", OP.bypass, ins=[ag_in[:]], outs=[tb[:]],
                    replica_groups=rg)

            # ---------- helper: gathers for one tile from a split table ----------
            def load_counts(t, n):
                nc.gpsimd.reg_load(gregs[:n], cnt[0:1, t * 8:t * 8 + n])

            def tile_gathers(gt, tb, t, width, qbase):
                for gi, (hf, ch0, nch) in enumerate(groups):
                    ni = nch * P
                    col0 = t * SCOLS + ch0 * 8
                    view = tb[0:SPLIT, :] if hf == 0 else tb[SPLIT:N_PAD, :]
                    nc.gpsimd.dma_gather(
                        gt[:, ch0:ch0 + nch, :], view,
                        sidx[:, col0:col0 + nch * 8], ni, gregs[gi], width,
                        queue_num=(qbase + gi) % 4)

            # ---------- helper: layernorm(+relu) on node-major tile ----------
            def ln_relu(dst, src, gam, bet):
                st = spool.tile([P, 6], f32, tag="lnst")
                nc.vector.bn_stats(out=st[:], in_=src[:])
                mv = spool.tile([P, 2], f32, tag="lnmv")
                nc.vector.bn_aggr(out=mv[:], in_=st[:])
                rstd = spool.tile([P, 1], f32, tag="lnrs")
                nc.scalar.activation(out=rstd[:], in_=mv[:, 1:2], func=AF.Sqrt,
                                     bias=eps_t[:], scale=1.0)
                nc.vector.reciprocal(out=rstd[:], in_=rstd[:])
                nc.vector.tensor_scalar(out=src[:], in0=src[:], scalar1=mv[:, 0:1],
                                        scalar2=rstd[:], op0=OP.subtract, op1=OP.mult)
                nc.vector.tensor_mul(out=src[:], in0=src[:], in1=gam[:])
                nc.vector.tensor_add(out=src[:], in0=src[:], in1=bet[:])
                nc.scalar.activation(out=dst[:], in_=src[:], func=AF.Relu)

            # ---------- helper: GCN aggregation pass ----------
            def gcn_pass(tb, bg_t, gam, bet, outT, resT):
                for t in range(TPC):
                    gt = gpool.tile([P, CPT, HID], bf16, tag="gcng")
                    load_counts(t, len(groups))
                    tile_gathers(gt, tb, t, HID, qbase=t)
                    U = pspool.tile([P, HID], f32, tag="U1")
                    for j in range(CPT):
                        c = t * CPT + j
                        oh = ohpool.tile([P, P], bf16, tag="gcnoh")
                        nc.vector.tensor_tensor(
                            out=oh[:], in0=dstslot[:, c:c + 1].to_broadcast([P, P]),
                            in1=iota_t[:], op=OP.is_equal)
                        nc.tensor.matmul(out=U[:], lhsT=oh[:], rhs=gt[:, j, :],
                                         start=(j == 0), stop=(j == CPT - 1))
                    pre = wpool.tile([P, HID], f32, tag="gcnpre")
                    nc.vector.scalar_tensor_tensor(
                        out=pre[:], in0=U[:], scalar=dinv_t[:, t:t + 1], in1=bg_t[:],
                        op0=OP.mult, op1=OP.add)
                    nm = wpool.tile([P, HID], bf16, tag="gcnnm")
                    ln_relu(nm, pre, gam, bet)
                    tp = tppool.tile([P, P], bf16, tag="tp")
                    nc.tensor.transpose(out=tp[:], in_=nm[:], identity=identbf[:])
                    if resT is None:
                        nc.vector.tensor_copy(out=outT[:, t * P:(t + 1) * P], in_=tp[:])
                    else:
                        nc.vector.tensor_add(out=outT[:, t * P:(t + 1) * P],
                                             in0=resT[:, t * P:(t + 1) * P], in1=tp[:])

            # ---------- GCN layer 1 ----------
            build_table(h0T, wg1_bf, ag_in1, tb1, True)
            x1T = bigpool.tile([P, NPC], bf16, tag="x1T")
            gcn_pass(tb1, bg1_t, g1g_t, g1b_t, x1T, None)

            # ---------- GCN layer 2 (residual) ----------
            build_table(x1T, wg2_bf, ag_in2, tb2, True)
            x2T = bigpool.tile([P, NPC], bf16, tag="h0T")  # reuse h0T slot
            gcn_pass(tb2, bg2_t, g2g_t, g2b_t, x2T, x1T)

            # ---------- P6: GAT table ----------
            for t in range(TPC):
                ps = pspool.tile([P, GD], f32, tag="mm")
                nc.tensor.matmul(out=ps[:], lhsT=x2T[:, t * P:(t + 1) * P],
                                 rhs=wgat_bf[:], start=True, stop=True)
                ps8 = tppool.tile([P, 2 * HEADS], f32, tag="tp")
                nc.tensor.matmul(out=ps8[:], lhsT=x2T[:, t * P:(t + 1) * P],
                                 rhs=vsvd_bf[:], start=True, stop=True)
                stg = wpool.tile([P, GROW], bf16, tag="stg")
                stg4 = stg[:, 0:HEADS * 129].rearrange("p (h c) -> p h c", h=HEADS)
                nc.vector.tensor_copy(
                    out=stg4[:, :, 0:P],
                    in_=ps[:].rearrange("p (h c) -> p h c", h=HEADS))
                nc.vector.memset(stg4[:, :, P:P + 1], 1.0)
                stgf = stg[:, AS_OFF:AS_OFF + 8].bitcast(f32)
                nc.vector.tensor_copy(out=stgf, in_=ps8[:, 0:HEADS])
                adsb = spool.tile([P, 8], bf16, tag="adsb")
                nc.vector.tensor_copy(out=adsb[:].bitcast(f32),
                                      in_=ps8[:, HEADS:2 * HEADS])
                nc.sync.dma_start(out=ad_d[t * P:(t + 1) * P, 0:8], in_=adsb[:])
                nc.sync.dma_start(out=ag_in3[t * P:(t + 1) * P, :], in_=stg[:])
            nc.gpsimd.collective_compute(
                "AllGather", OP.bypass, ins=[ag_in3[:]], outs=[tb3[:]],
                replica_groups=rg)

            # ---------- P7: GAT aggregation + att_out ----------
            aoutT = bigpool.tile([P, NPC], bf16, tag="x1T")  # reuse x1T slot
            for t in range(TPC):
                gt = gpool.tile([P, CPT, GROW], bf16, tag="gatg")
                load_counts(t, NG)
                tile_gathers(gt, tb3, t, GROW, qbase=t)
                ad = adpool.tile([P, CPT, ADW], bf16, tag="adg")
                for gi, (ch0, nch) in enumerate(dgroups):
                    ni = nch * P
                    col0 = t * SCOLS + ch0 * 8
                    nc.gpsimd.dma_gather(
                        ad[:, ch0:ch0 + nch, :], ad_d[:],
                        didx[:, col0:col0 + nch * 8], ni,
                        gregs[len(groups) + gi], ADW,
                        queue_num=(t + gi + 2) % 4)

                # batched eat = leaky_relu(a_s + a_d); ex = exp(eat)
                ea = spool.tile([P, CPT, HEADS], f32, tag="ea")
                nc.vector.tensor_tensor(
                    out=ea[:], in0=gt[:, :, AS_OFF:AS_OFF + 8].bitcast(f32),
                    in1=ad[:, :, 0:8].bitcast(f32), op=OP.add)
                eaf = ea[:].rearrange("p a b -> p (a b)")
                nc.vector.scalar_tensor_tensor(
                    out=eaf, in0=eaf, scalar=0.2, in1=eaf,
                    op0=OP.mult, op1=OP.max)
                ex = spool.tile([P, CPT, HEADS], bf16, tag="ex")
                nc.scalar.activation(out=ex[:].rearrange("p a b -> p (a b)"),
                                     in_=eaf, func=AF.Exp)

                U12 = pspool.tile([P, 258], f32, tag="U1")
                U34 = pspool.tile([P, 258], f32, tag="U2")
                for j in range(CPT):
                    c = t * CPT + j
                    ohw = ohpool.tile([P, HEADS, P], bf16, tag="gatohw")
                    nc.vector.scalar_tensor_tensor(
                        out=ohw[:], in0=iota4[:],
                        scalar=dstslot[:, c:c + 1],
                        in1=ex[:, j, :].to_broadcast([P, HEADS, P]),
                        op0=OP.is_equal, op1=OP.mult)
                    for h in range(HEADS):
                        Ub = U12 if h < 2 else U34
                        pos = (h % 2) * 129
                        nc.tensor.matmul(
                            out=Ub[:, pos:pos + 129], lhsT=ohw[:, h, :],
                            rhs=gt[:, j, h * 129:(h + 1) * 129],
                            start=(j == 0), stop=(j == CPT - 1))

                den4 = spool.tile([P, HEADS], f32, tag="den4")
                for h in range(HEADS):
                    Ub = U12 if h < 2 else U34
                    pos = (h % 2) * 129 + 128
                    nc.scalar.activation(out=den4[:, h:h + 1],
                                         in_=Ub[:, pos:pos + 1], func=AF.Copy)
                rden = spool.tile([P, HEADS], f32, tag="rden")
                nc.vector.reciprocal(out=rden[:], in_=den4[:])
                gat = wpool.tile([P, GD], bf16, tag="gat")
                for h in range(HEADS):
                    Ub = U12 if h < 2 else U34
                    pos = (h % 2) * 129
                    nc.scalar.activation(
                        out=gat[:, h * P:(h + 1) * P], in_=Ub[:, pos:pos + P],
                        func=AF.Copy, scale=rden[:, h:h + 1])
                ao = pspool.tile([P, P], f32, tag="mm")
                for k in range(4):
                    tp = tppool.tile([P, P], bf16, tag="tp")
                    nc.tensor.transpose(out=tp[:], in_=gat[:, k * P:(k + 1) * P],
                                        identity=identbf[:])
                    aT = wpool.tile([P, P], bf16, tag="aT")
                    nc.vector.tensor_copy(out=aT[:], in_=tp[:])
                    nc.tensor.matmul(out=ao[:], lhsT=watt_bf[:, k, :], rhs=aT[:],
                                     start=(k == 0), stop=(k == 3))
                nc.scalar.activation(out=aoutT[:, t * P:(t + 1) * P], in_=ao[:],
                                     func=AF.Relu, bias=batt_t[:], scale=1.0)

            # ---------- P8: output projection ----------
            for t in range(TPC):
                po = pspool.tile([P, OUT_DIM], f32, tag="mm")
                nc.tensor.matmul(out=po[:], lhsT=aoutT[:, t * P:(t + 1) * P],
                                 rhs=wout_bf[:], start=True, stop=True)
                osb = wpool.tile([P, OUT_DIM], f32, tag="osb")
                nc.vector.tensor_add(out=osb[:], in0=po[:], in1=bout_t[:])
                nc.sync.dma_start(out=out_c[t * P:(t + 1) * P, :], in_=osb[:])

    nc.finalize()
    return nc


_CACHE = {}


def kernel(_trace=False, **inputs):
    from concourse import bass_utils

    ei = np.asarray(inputs["edge_index"])
    sidx16, didx16, dst_slot, dinv_new, perm, plan, counts = _preprocess(ei)
    CPT = plan["CPT"]
    CHUNKS = TPC * CPT

    x = np.asarray(inputs["x"], dtype=np.float32)
    xP = np.zeros((N_PAD, IN_DIM), np.float32)
    xP[perm[:N]] = x

    g = lambda k: np.asarray(inputs[k], dtype=np.float32)
    Wgat = g("Wgat")
    Wg3 = Wgat.reshape(HID, HEADS, HID)
    Vs = np.einsum("khc,hc->kh", Wg3, g("att_src")).astype(np.float32)
    Vd = np.einsum("khc,hc->kh", Wg3, g("att_dst")).astype(np.float32)
    batt_f = (g("bgat") @ g("Watt_out") + g("batt_out")).astype(np.float32)

    bc = lambda v, w: np.tile(np.asarray(v, np.float32)[None, :w], (P, 1))

    common = {
        "iota_f": np.tile(np.arange(P, dtype=np.float32)[None, :], (P, 1)),
        "ident": np.eye(P, dtype=np.float32),
        "Win": g("Win"),
        "bin_pp": g("bin_")[:, None].astype(np.float32),
        "Wg1": g("Wg1"), "Wg2": g("Wg2"),
        "bg1_bc": bc(g("bg1"), HID), "bg2_bc": bc(g("bg2"), HID),
        "g1g_bc": bc(g("g1_gamma"), HID), "g1b_bc": bc(g("g1_beta"), HID),
        "g2g_bc": bc(g("g2_gamma"), HID), "g2b_bc": bc(g("g2_beta"), HID),
        "Wgat": Wgat,
        "VsVd": np.concatenate([Vs, Vd], axis=1).astype(np.float32),
        "Watt": g("Watt_out"),
        "batt_pp": batt_f[:, None].astype(np.float32),
        "Wout": g("Wout"),
        "bout_bc": bc(g("bout"), OUT_DIM),
        "eps_pp": np.full((P, 1), 1e-5, np.float32),
    }

    key = ("nc", plan["cA"], plan["cB"])
    if key not in _CACHE:
        _CACHE[key] = _build_nc(plan)
    nc = _CACHE[key]

    in_maps = []
    for c in range(NCORES):
        m = dict(common)
        m["x_c"] = np.ascontiguousarray(xP[c * NPC:(c + 1) * NPC])
        m["sidx16"] = np.ascontiguousarray(sidx16[c])
        m["didx16"] = np.ascontiguousarray(didx16[c])
        m["counts"] = np.tile(counts[c][None, :], (P, 1)).astype(np.int32)
        m["dstslot"] = np.ascontiguousarray(
            dst_slot[c].reshape(CHUNKS, P).T).astype(np.float32)
        m["dinv_t"] = np.ascontiguousarray(
            dinv_new[c * NPC:(c + 1) * NPC].reshape(TPC, P).T).astype(np.float32)
        in_maps.append(m)

    res = bass_utils.run_bass_kernel_spmd(
        nc, in_maps, core_ids=list(range(NCORES)), trace=_trace)

    outP = np.concatenate([res.results[c]["out_c"] for c in range(NCORES)], axis=0)
    out = outP[perm[:N]]
    if _trace:
        kernel._last_exec_ns = res.exec_time_ns
    return out.astype(np.float32)
